# revision 17
# baseline (speedup 1.0000x reference)
"""Trainium2 Bass kernel for nn_Attention_Joint_MaxPool.

Math (see reference):
  q = (Wq*scale) @ x                        (B, C, N), heads on rows
  xsr = conv2x2s2(x) ; k = Wk @ BN(xsr)     (B, C, Nk=1024)
  attn = max over keys of q_h . k_h         (B, NH, N)
  s = sum over heads of attn                (B, N)
  out[b,c,n] = (Wproj @ mean_n x)[c] * s[b,n] + bproj[c]

Weight folding done on host:
  g = gamma/sqrt(var+eps); A = Wk * g[None,:]
  k = sum_e (A @ Wsr[:,:,e]) @ x_sub[e] + ck,  ck = A@bsr + Wk@(beta-mean*g)
  pv[b] = Wproj @ mean_n x[b]   (rank-1 output structure)

Key structure (v2):
  All heavy matmuls run in bf16.  Max over keys via the pair cascade
  max(a,b) = a + relu(b-a): diff-pair matmuls write a PSUM bank, ScalarE
  relus it IN PLACE (has_written survives), even-pair matmuls accumulate
  onto it with start=False, VectorE reduce_maxes the bank.  The two heads
  of a pair run CONCURRENTLY in the PE via tile_position row pairing
  (measured: second matmul of a pair retires ~4ns after the first).

  Scheduling (the v2 changes):
  - all input DMAs ride ONE HWDGE ring in strict priority order
    (cpb -> wq+xq0+wk01 -> convA -> xq123 -> convB -> wk23) so the
    k-path data lands ~21us in instead of ~28us (round-robin starved).
  - PE warmup burst on memset junk so HAM is at 8/8 before real work.
  - k m1 accumulates in a borrowed score-pool PSUM tile during the lead.
  - passes: A (m=0), B (m=1), then CD (m=2,3 interleaved per tile), all
    with a 2-deep front/finish software pipeline so the ScalarE relu
    latency never head-of-line-blocks the PE.
  - k m2/m3 matmuls and the q-projection units are spread across pass
    A/B slots with explicit data deadlines.
  - output stage: 8 parts of 2 token tiles each, emitted as soon as the
    pair of tiles finalizes in pass CD; output DMAs on the scalar ring.

Sharding: 8 cores; core i -> batch i//2, token half i%2 (2048 tokens).
Each core is fully independent (no collectives).
"""

import os
import sys
import types
import numpy as np

# ---------------------------------------------------------------------------
# problem constants (hardcoded; kernel.py must be self-contained)
# ---------------------------------------------------------------------------
B, C, N = 4, 512, 4096
NH, HD = 8, 64
SR = 2
EPS = 1e-5
HW_ = 64                      # H = W = 64
T = N // 2                    # tokens per core
NK = 1024                     # conv output positions (keys)
NKE = NK // 2                 # even keys
MB = C // 128                 # 4 channel blocks
KC = C // 128                 # 4 contraction chunks
NCORES = 8
TT = T // 128                 # 16 token tiles per core
NCH = T // 512                # 4 q chunks per core

_cache = {}


# ---------------------------------------------------------------------------
# workarounds for this container's toolchain
# ---------------------------------------------------------------------------
def _install_fixes():
    import concourse.tile as tile
    import concourse.mybir as mybir
    from concourse.vector_clock import ScopedClock

    if getattr(tile.TileContext, "_drain_patched", False):
        return

    def _patched_drain_and_barrier(self, tick_clock, wait_clock):
        nc = self.nc
        probe = nc.sync.nop(nofuse=True, hint="drain_wait_carrier")
        wait_clock.add_sem_waits(
            probe.ins, ScopedClock({None: tick_clock.global_clock})
        )
        waits = list(probe.ins.sync_info.on_wait) if probe.ins.sync_info else []
        if len(waits) > 1:
            probe.ins.sync_info = mybir.SyncInfo(on_wait=waits[:1], on_update=[])
            for w in waits[1:]:
                extra = nc.sync.nop(nofuse=True, hint="drain_wait_carrier")
                extra.ins.sync_info = mybir.SyncInfo(on_wait=[w], on_update=[])
        nc.sync.drain()
        nc.all_engine_barrier()
        assert self.sems is not None
        popped = nc._tile_sem_poison_stack.pop()
        assert popped is self._sem_poison
        nc.clear_and_free_semaphores(list(self.sems.allocated().values()))
        nc.all_engine_barrier()

    tile.TileContext._drain_and_barrier = _patched_drain_and_barrier
    tile.TileContext._drain_patched = True


def _split_multi_waits(nc):
    """This walrus build allows only one sync-wait per instruction; hoist
    extra waits onto same-engine nops inserted just before the instruction."""
    import concourse.mybir as mybir

    ctr = 0
    for f in nc.m.functions:
        for bb in f.blocks:
            changed = False
            out = []
            for inst in bb.instructions:
                si = inst.sync_info
                tname = type(inst).__name__
                if (si is not None and si.on_wait and len(si.on_wait) > 1
                        and "Collective" not in tname):
                    waits = list(si.on_wait)
                    for w in waits[:-1]:
                        ctr += 1
                        nop = mybir.InstNoOp(
                            name=f"I-ws-{ctr}",
                            engine=inst.engine,
                            sync_info=mybir.SyncInfo(on_wait=[w], on_update=[]),
                        )
                        nc.register_instruction(nop, overwrite=True)
                        out.append(nop)
                    inst.sync_info = mybir.SyncInfo(
                        on_wait=waits[-1:], on_update=list(si.on_update)
                    )
                    changed = True
                out.append(inst)
            if changed:
                bb.instructions = out


def _install_ntff_hook():
    """Provide antenv.axon_hooks (missing in this image) so trace=True works."""
    try:
        from antenv import axon_hooks  # noqa: F401
        return
    except ImportError:
        pass
    try:
        import antenv
        from trn_agent_boot.trn_boot import _ntff_profile_via_ctypes
    except ImportError:
        return
    mod = types.ModuleType("antenv.axon_hooks")
    _hook = [None]
    mod.set_axon_ntff_profile_hook = lambda h: _hook.__setitem__(0, h)
    mod.get_axon_ntff_profile_hook = lambda: _hook[0]
    sys.modules["antenv.axon_hooks"] = mod
    antenv.axon_hooks = mod
    mod.set_axon_ntff_profile_hook(
        _ntff_profile_via_ctypes("/opt/axon/libaxon_pjrt.so")
    )


# ---------------------------------------------------------------------------
# device program
# ---------------------------------------------------------------------------
def _build_program():
    import concourse.bass as bass
    import concourse.mybir as mybir
    import concourse.tile as tile

    F32 = mybir.dt.float32
    F32R = mybir.dt.float32r
    BF16 = mybir.dt.bfloat16
    AX = mybir.AxisListType
    ACTF = mybir.ActivationFunctionType
    ALU = mybir.AluOpType

    nc = bass.Bass()

    cpbx_in = nc.declare_dram_parameter("cpbx", [128, 3 * MB], F32,
                                        isOutput=False)
    ones_in = nc.declare_dram_parameter("ones", [1, 128], F32R,
                                        isOutput=False)
    pack1_in = nc.declare_dram_parameter("pack1", [128, 8192], BF16,
                                         isOutput=False)
    convA_in = nc.declare_dram_parameter("convA", [128, 8192], BF16,
                                         isOutput=False)
    xq123_in = nc.declare_dram_parameter("xq123", [128, 6144], BF16,
                                         isOutput=False)
    convB_in = nc.declare_dram_parameter("convB", [128, 8192], BF16,
                                         isOutput=False)
    wk23_in = nc.declare_dram_parameter("wk23", [128, 4096], BF16,
                                        isOutput=False)
    out_ext = nc.declare_dram_parameter("out", [C, T], F32, isOutput=True)

    sbounce = nc.dram_tensor("sbounce", [128, TT], F32)

    # output parts: (lo_tt, hi_tt); big parts early, small parts for the tail
    PARTS = [(0, 4), (4, 8), (8, 12), (12, 14), (14, 16)]

    with tile.TileContext(nc) as tc:
        with tc.tile_pool(name="wts", bufs=1) as wts, \
             tc.tile_pool(name="xdat", bufs=1) as xdat, \
             tc.tile_pool(name="work", bufs=1) as work, \
             tc.tile_pool(name="opool", bufs=2) as opool, \
             tc.tile_pool(name="psX", bufs=3, space="PSUM") as psX, \
             tc.tile_pool(name="pkp", bufs=1, space="PSUM") as pkp:

            # ---- input DMAs ----
            # big blobs: strict priority chain on the SP HWDGE ring
            pack1 = wts.tile([128, 8192], BF16, tag="pack1")
            nc.sync.dma_start(out=pack1[:], in_=pack1_in[:])
            convA = xdat.tile([128, 8192], BF16, tag="convA")
            nc.sync.dma_start(out=convA[:], in_=convA_in[:])
            xq123 = wts.tile([128, 6144], BF16, tag="xq123")
            nc.sync.dma_start(out=xq123[:], in_=xq123_in[:])
            convB = xdat.tile([128, 8192], BF16, tag="convB")
            nc.sync.dma_start(out=convB[:], in_=convB_in[:])
            wk23 = xdat.tile([128, 4096], BF16, tag="wk23")
            nc.sync.dma_start(out=wk23[:], in_=wk23_in[:])
            # small constants ride the ACT ring in parallel
            cpbx_t = wts.tile([128, 3 * MB], F32, tag="cpbx")
            nc.scalar.dma_start(out=cpbx_t[:], in_=cpbx_in[:])
            ones = wts.tile([1, 128], F32R, tag="ones")
            nc.scalar.dma_start(out=ones[:], in_=ones_in[:])

            ck_t = cpbx_t[:, 0:MB]
            pv_t = cpbx_t[:, MB:2 * MB]
            bb_t = cpbx_t[:, 2 * MB:3 * MB]

            # warmup junk (memset; no DMA dependency)
            warm = wts.tile([128, 128], BF16, tag="warm")
            nc.gpsimd.memset(warm[:], 0.0)

            # views
            wq_t = [pack1[:, kc * 512:(kc + 1) * 512] for kc in range(KC)]
            xq_t = {}
            for kc in range(KC):
                xq_t[(0, kc)] = pack1[:, 2048 + kc * 512:2048 + (kc + 1) * 512]
                for c in range(1, NCH):
                    xq_t[(c, kc)] = xq123[:, (c - 1) * 2048 + kc * 512:
                                          (c - 1) * 2048 + (kc + 1) * 512]
            xce_t, xcd_t, wksr_t = {}, {}, {}
            for kc in range(KC):
                blob = convA if kc < 2 else convB
                base = (kc % 2) * 4096
                for e in range(4):
                    xce_t[(e, kc)] = blob[:, base + e * 512:
                                          base + (e + 1) * 512]
                    xcd_t[(e, kc)] = blob[:, base + 2048 + e * 512:
                                          base + 2048 + (e + 1) * 512]
                for m in range(MB):
                    if m < 2:
                        wksr_t[(m, kc)] = pack1[:, 4096 + m * 2048 + kc * 512:
                                                4096 + m * 2048 + (kc + 1) * 512]
                    else:
                        wksr_t[(m, kc)] = wk23[:, (m - 2) * 2048 + kc * 512:
                                               (m - 2) * 2048 + (kc + 1) * 512]

            # ---- persistent activations ----
            q_sb = [work.tile([128, T], BF16, tag=f"q{m}", name=f"q{m}")
                    for m in range(MB)]
            k2_sb = [work.tile([128, NK], BF16, tag=f"k2{m}", name=f"k2{m}")
                     for m in range(MB)]
            s_acc = work.tile([128, TT * NH], F32, tag="sacc")
            s_cols = work.tile([128, TT], F32, tag="scols")
            sflat = work.tile([1, T], F32R, tag="sflat")

            # ---- q projection unit for one (chunk, head-pair) ----
            def emit_q_unit(c, m, eng="v"):
                pq = psX.tile([128, 1024], F32, tag="xbank",
                              name=f"pq{c}_{m}")
                for kc in range(KC):
                    nc.tensor.matmul(
                        pq[:, 0:512],
                        wq_t[kc][:, m * 128:(m + 1) * 128],
                        xq_t[(c, kc)],
                        start=(kc == 0), stop=(kc == KC - 1))
                dst = q_sb[m][:, c * 512:(c + 1) * 512]
                # engine split chosen to balance Scalar vs Vector per phase
                if eng == "s":
                    nc.scalar.copy(dst, pq[:, 0:512])
                else:
                    nc.vector.tensor_copy(dst, pq[:, 0:512])

            # ---- k even/diff banks for head pair m ----
            def emit_k_mms(m, pk, units):
                for (e, kc) in units:
                    first = (kc == 0 and e == 0)
                    last = (kc == KC - 1 and e == 3)
                    nc.tensor.matmul(
                        pk[:, 0:512],
                        wksr_t[(m, kc)][:, e * 128:(e + 1) * 128],
                        xce_t[(e, kc)],
                        start=first, stop=last)
                    nc.tensor.matmul(
                        pk[:, 512:1024],
                        wksr_t[(m, kc)][:, e * 128:(e + 1) * 128],
                        xcd_t[(e, kc)],
                        start=first, stop=last)

            def emit_k_act(m, pk):
                nc.scalar.activation(
                    k2_sb[m][:, 0:512], pk[:, 0:512], ACTF.Identity,
                    bias=ck_t[:, m:m + 1], scale=1.0)
                nc.scalar.copy(k2_sb[m][:, 512:1024], pk[:, 512:1024])

            # ---- score groups: front (diff+relu) / finish (even+reduce) ----
            state = {}

            def emit_front(m, tt):
                tsl = slice(tt * 128, (tt + 1) * 128)
                qs = q_sb[m]
                pX = psX.tile([128, 1024], F32, tag="xbank",
                              name=f"pX{m}_{tt}")
                nc.tensor.matmul(pX[:, 0:512], qs[0:64, tsl],
                                 k2_sb[m][0:64, 512:1024], start=True,
                                 stop=True, tile_position=(0, 0))
                nc.tensor.matmul(pX[:, 512:1024], qs[64:128, tsl],
                                 k2_sb[m][64:128, 512:1024], start=True,
                                 stop=True, tile_position=(64, 0))
                nc.scalar.activation(pX[:], pX[:], ACTF.Relu)
                state[(m, tt)] = pX

            def emit_finish(m, tt):
                tsl = slice(tt * 128, (tt + 1) * 128)
                qs = q_sb[m]
                pX = state.pop((m, tt))
                nc.tensor.matmul(pX[:, 0:512], qs[0:64, tsl],
                                 k2_sb[m][0:64, 0:512], start=False,
                                 stop=True, tile_position=(0, 0))
                nc.tensor.matmul(pX[:, 512:1024], qs[64:128, tsl],
                                 k2_sb[m][64:128, 0:512], start=False,
                                 stop=True, tile_position=(64, 0))
                cols = slice(tt * NH + 2 * m, tt * NH + 2 * m + 2)
                nc.vector.reduce_max(
                    s_acc[:, cols],
                    pX[:].rearrange("p (a b) -> p a b", a=2), axis=AX.X)
                if m == MB - 1:
                    nc.vector.reduce_sum(
                        s_cols[:, tt:tt + 1],
                        s_acc[:, tt * NH:(tt + 1) * NH], axis=AX.X)

            # ---- rank-1 output stage, one part = PARTS[p] token tiles ----
            # s_cols -> DRAM bounce -> token-major sflat (GpSimd gather),
            # then a ones-matmul broadcasts s over the 128 channel
            # partitions.  The four steps are staged one score-group apart
            # so the DMA latency never head-of-line-blocks the PE.
            out_state = {}

            def outer_g1(p):
                lo_tt, hi_tt = PARTS[p]
                with nc.named_scope("outer"):
                    nc.sync.dma_start(out=sbounce[:, lo_tt:hi_tt],
                                      in_=s_cols[:, lo_tt:hi_tt])

            def outer_g2(p):
                lo_tt, hi_tt = PARTS[p]
                with nc.named_scope("outer"):
                    nc.gpsimd.dma_start(
                        out=sflat[0:1, lo_tt * 128:hi_tt * 128],
                        in_=sbounce[:, lo_tt:hi_tt].rearrange(
                            "p t -> () t p"))

            def outer_mm(p):
                lo_tt, hi_tt = PARTS[p]
                ntok = (hi_tt - lo_tt) * 128
                with nc.named_scope("outer"):
                    pbc = pkp.tile([128, 1024], F32, tag="kbank",
                                   name=f"pbc{p}")
                    nc.tensor.matmul(
                        pbc[:, 0:ntok], ones[:],
                        sflat[0:1, lo_tt * 128:hi_tt * 128],
                        start=True, stop=True)
                    out_state[p] = pbc

            def outer_acts(p):
                lo_tt, hi_tt = PARTS[p]
                ntok = (hi_tt - lo_tt) * 128
                tok = slice(lo_tt * 128, hi_tt * 128)
                pbc = out_state.pop(p)
                last = (p == len(PARTS) - 1)
                with nc.named_scope("outer"):
                    osb = opool.tile([128, 4 * 512], F32, tag="osb",
                                     name=f"osb{p}")
                    for m in range(MB):
                        if last and m % 2 == 1:
                            nc.vector.tensor_scalar(
                                osb[:, m * ntok:(m + 1) * ntok],
                                pbc[:, 0:ntok],
                                pv_t[:, m:m + 1], bb_t[:, m:m + 1],
                                op0=ALU.mult, op1=ALU.add)
                        else:
                            nc.scalar.activation(
                                osb[:, m * ntok:(m + 1) * ntok],
                                pbc[:, 0:ntok], ACTF.Identity,
                                bias=bb_t[:, m:m + 1], scale=pv_t[:, m:m + 1])
                    nc.scalar.dma_start(
                        out=out_ext[:, tok].rearrange("(m p) t -> p m t",
                                                      m=MB),
                        in_=osb[:, 0:MB * ntok].rearrange(
                            "p (m t) -> p m t", m=MB))

            # ------------------ emission schedule ------------------
            units_of = lambda kcs: [(e, kc) for kc in kcs for e in range(4)]

            # warmup: ~36 junk MMs to trip HAM to 8/8 before real work
            pkw = pkp.tile([128, 1024], F32, tag="kbank", name="pkwarm")
            with nc.named_scope("warm"):
                for i in range(36):
                    nc.tensor.matmul(pkw[:, 0:128], warm[:], warm[:],
                                     start=True, stop=True)

            # lead
            with nc.named_scope("lead"):
                for m in range(MB):
                    emit_q_unit(0, m)          # pack1 (q chunk 0, all pairs)
                pk0 = pkp.tile([128, 1024], F32, tag="kbank", name="pk0")
                xbm1 = psX.tile([128, 1024], F32, tag="xbank", name="xbm1")
                # convA: kc0/1 for m0 and m1, interleaved
                for kc in (0, 1):
                    emit_k_mms(0, pk0, units_of([kc]))
                    emit_k_mms(1, xbm1, units_of([kc]))
                emit_q_unit(1, 0)              # xq123
                emit_q_unit(1, 1)
                # convB: m1 first so k_act(1) frees xbm1's pool slot before
                # pass A's third front needs it
                for kc in (2, 3):
                    emit_k_mms(1, xbm1, units_of([kc]))
                emit_k_act(1, xbm1)
                for kc in (2, 3):
                    emit_k_mms(0, pk0, units_of([kc]))
                emit_k_act(0, pk0)

            # score passes with 2-deep pipeline
            pending = []

            def do_group(m, tt):
                emit_front(m, tt)
                pending.append((m, tt))
                if len(pending) > 2:
                    emit_finish(*pending.pop(0))

            def drain():
                while pending:
                    emit_finish(*pending.pop(0))

            # pass A: m=0; fillers: k m2 (kc0..3), q units (scalar casts:
            # ScalarE has slack in A/B, VectorE is the tighter engine)
            pkA = pkp.tile([128, 1024], F32, tag="kbank", name="pk2")
            fillA = {
                1: lambda: emit_k_mms(2, pkA, units_of([0])[0:2]),
                2: lambda: emit_k_mms(2, pkA, units_of([0])[2:4]),
                3: lambda: emit_q_unit(2, 0, "s"),
                4: lambda: emit_k_mms(2, pkA, units_of([1])[0:2]),
                5: lambda: emit_k_mms(2, pkA, units_of([1])[2:4]),
                6: lambda: emit_q_unit(3, 0, "s"),
                7: lambda: emit_k_mms(2, pkA, units_of([2])[0:2]),
                8: lambda: emit_k_mms(2, pkA, units_of([2])[2:4]),
                9: lambda: emit_q_unit(1, 2, "s"),
                10: lambda: emit_k_mms(2, pkA, units_of([3])[0:2]),
                11: lambda: (emit_k_mms(2, pkA, units_of([3])[2:4]),
                             emit_k_act(2, pkA)),
            }
            with nc.named_scope("passA"):
                for tt in range(TT):
                    do_group(0, tt)
                    f = fillA.get(tt)
                    if f:
                        f()

            # pass B: m=1; fillers: k m3 + the pass-B-deadline q units
            pkB = pkp.tile([128, 1024], F32, tag="kbank", name="pk3")
            fillB = {
                1: lambda: emit_k_mms(3, pkB, units_of([0])[0:2]),
                2: lambda: emit_k_mms(3, pkB, units_of([0])[2:4]),
                3: lambda: emit_q_unit(2, 1, "s"),
                4: lambda: emit_k_mms(3, pkB, units_of([1])[0:2]),
                5: lambda: emit_k_mms(3, pkB, units_of([1])[2:4]),
                6: lambda: emit_q_unit(3, 1, "s"),
                7: lambda: emit_k_mms(3, pkB, units_of([2])[0:2]),
                8: lambda: emit_k_mms(3, pkB, units_of([2])[2:4]),
                9: lambda: emit_q_unit(1, 3, "s"),
                10: lambda: emit_k_mms(3, pkB, units_of([3])[0:2]),
                11: lambda: (emit_k_mms(3, pkB, units_of([3])[2:4]),
                             emit_k_act(3, pkB)),
            }
            with nc.named_scope("passB"):
                for tt in range(TT):
                    do_group(1, tt)
                    f = fillB.get(tt)
                    if f:
                        f()

            # pass CD: m=2,3 interleaved per tile; output-part work is queued
            # at each part's final finish and drained one action per group so
            # nothing bunches up or head-of-line-blocks the PE.
            fin_hooks = {}
            actions = []

            def do_group_cd(m, tt):
                emit_front(m, tt)
                pending.append((m, tt))
                if len(pending) > 2:
                    g = pending.pop(0)
                    emit_finish(*g)
                    h = fin_hooks.pop(g, None)
                    if h:
                        actions.extend(h)
                if actions:
                    actions.pop(0)()

            with nc.named_scope("passCD"):
                for p in range(len(PARTS)):
                    fin_hooks[(3, PARTS[p][1] - 1)] = [
                        (lambda pp: lambda: outer_g1(pp))(p),
                        (lambda pp: lambda: outer_g2(pp))(p),
                        (lambda pp: lambda: outer_mm(pp))(p),
                        (lambda pp: lambda: outer_acts(pp))(p),
                    ]
                qcd = {0: (2, 2), 2: (2, 3), 6: (3, 2), 8: (3, 3)}
                for tt in range(TT):
                    do_group_cd(2, tt)
                    if tt in qcd:
                        emit_q_unit(*qcd[tt])
                    do_group_cd(3, tt)
                while pending:
                    g = pending.pop(0)
                    emit_finish(*g)
                    h = fin_hooks.pop(g, None)
                    if h:
                        actions.extend(h)
                    if actions:
                        actions.pop(0)()
                while actions:
                    actions.pop(0)()

    _split_multi_waits(nc)
    return nc


# ---------------------------------------------------------------------------
# host side
# ---------------------------------------------------------------------------
def _prep_host(x, Wq, Wk, Wsr, bsr, bn_gamma, bn_beta, bn_mean, bn_var,
               Wproj, bproj):
    import ml_dtypes
    bf16 = ml_dtypes.bfloat16
    f8 = np.float64
    scale = HD ** -0.5
    g = bn_gamma.astype(f8) / np.sqrt(bn_var.astype(f8) + EPS)
    A = Wk.astype(f8) * g[None, :]
    ck = A @ bsr.astype(f8) + Wk.astype(f8) @ (
        bn_beta.astype(f8) - bn_mean.astype(f8) * g)
    # k weights: wk4[e] = (A @ Wsr[:,:,e]).T   (C_in, C_out)
    wk4 = np.stack([
        (A @ Wsr[:, :, e // 2, e % 2].astype(f8)).T for e in range(4)
    ])
    # wkblk[m][kc][e] = wk4[e][kc*128:(kc+1)*128, m*128:(m+1)*128]
    def wkblk(m):
        cols = np.empty((128, 2048), np.float64)
        for kc in range(KC):
            for e in range(4):
                cols[:, kc * 512 + e * 128:kc * 512 + (e + 1) * 128] = \
                    wk4[e][kc * 128:(kc + 1) * 128, m * 128:(m + 1) * 128]
        return cols

    wqT = (Wq.astype(f8) * scale).T                    # (C_in, C_out)

    x4 = x.reshape(B, C, HW_, HW_)
    xce = np.empty((B, C, 4, NKE), np.float32)
    xcd = np.empty((B, C, 4, NKE), np.float32)
    for e in range(4):
        di, dj = e // 2, e % 2
        even = x4[:, :, di::2, dj::4].reshape(B, C, NKE)
        odd = x4[:, :, di::2, dj + 2::4].reshape(B, C, NKE)
        xce[:, :, e] = even
        xcd[:, :, e] = odd - even
    xce = xce.reshape(B, C, 4 * NKE)
    xcd = xcd.reshape(B, C, 4 * NKE)
    # convA/convB [128, 8192]: [kc-of-pair][even|diff][e][512]
    convA = np.empty((B, 128, 8192), np.float32)
    convB = np.empty((B, 128, 8192), np.float32)
    for kc in range(KC):
        blob = convA if kc < 2 else convB
        base = (kc % 2) * 4096
        rows = slice(kc * 128, (kc + 1) * 128)
        blob[:, :, base:base + 2048] = xce[:, rows]
        blob[:, :, base + 2048:base + 4096] = xcd[:, rows]
    convA = convA.astype(bf16)
    convB = convB.astype(bf16)

    v = x.astype(f8).mean(axis=2)                       # (B, C)
    pv = (Wproj.astype(f8) @ v.T).T.astype(np.float32)  # (B, C)

    ck_t = ck.astype(np.float32).reshape(MB, 128).T    # (128, MB)
    bb_t = bproj.astype(np.float32).reshape(MB, 128).T
    cpbx = [np.concatenate(
        [ck_t, pv[b].reshape(MB, 128).T, bb_t], axis=1).astype(np.float32)
        for b in range(B)]                              # (128, 3*MB)

    # pack1 [128, 8192]: wq kc-major (2048) | xq chunk0 (2048) | wk m0,m1
    # xq123 [128, 6144]: [c-1][kc][512]
    # wk23  [128, 4096]: [m-2][kc][e][128]
    pack1 = np.empty((B, 2, 128, 8192), np.float32)
    xq123 = np.empty((B, 2, 128, 6144), np.float32)
    for half in range(2):
        xh = x[:, :, half * T:(half + 1) * T]
        for kc in range(KC):
            rows = slice(kc * 128, (kc + 1) * 128)
            pack1[:, half, :, kc * 512:(kc + 1) * 512] = wqT[None, rows]
            pack1[:, half, :, 2048 + kc * 512:2048 + (kc + 1) * 512] = \
                xh[:, rows, 0:512]
            for c in range(1, NCH):
                xq123[:, half, :, (c - 1) * 2048 + kc * 512:
                      (c - 1) * 2048 + (kc + 1) * 512] = \
                    xh[:, rows, c * 512:(c + 1) * 512]
    pack1[:, :, :, 4096:6144] = wkblk(0)[None, None]
    pack1[:, :, :, 6144:8192] = wkblk(1)[None, None]
    wk23 = np.concatenate([wkblk(2), wkblk(3)], axis=1).astype(bf16)
    pack1 = pack1.astype(bf16)
    xq123 = xq123.astype(bf16)
    return cpbx, pack1, convA, xq123, convB, wk23


def kernel(x, y, Wq, Wk, Wsr, bsr, bn_gamma, bn_beta, bn_mean, bn_var,
           Wproj, bproj, H, W):
    x = np.asarray(x, np.float32)
    cpbx, pack1, convA, xq123, convB, wk23 = _prep_host(
        x, np.asarray(Wq, np.float32), np.asarray(Wk, np.float32),
        np.asarray(Wsr, np.float32), np.asarray(bsr, np.float32),
        np.asarray(bn_gamma, np.float32), np.asarray(bn_beta, np.float32),
        np.asarray(bn_mean, np.float32), np.asarray(bn_var, np.float32),
        np.asarray(Wproj, np.float32), np.asarray(bproj, np.float32))

    _install_fixes()
    _install_ntff_hook()
    from concourse.bass_utils import run_bass_kernel_spmd

    if "nc" not in _cache:
        _cache["nc"] = _build_program()
    nc = _cache["nc"]

    in_maps = []
    for core in range(NCORES):
        b, half = core // 2, core % 2
        in_maps.append({
            "cpbx": cpbx[b],
            "pack1": pack1[b, half],
            "convA": convA[b],
            "xq123": xq123[b, half],
            "convB": convB[b],
            "wk23": wk23,
            "ones": np.ones((1, 128), np.float32),
        })

    trace = os.environ.get("BASS_KERNEL_TRACE", "0") == "1"
    res = run_bass_kernel_spmd(nc, in_maps, list(range(NCORES)), trace=trace)
    if trace:
        print(f"HW exec time: {res.exec_time_ns} ns")
        _cache["last_exec_time_ns"] = res.exec_time_ns
        _cache["last_trace"] = res.instructions_and_trace

    out = np.empty((B, C, N), np.float32)
    for core in range(NCORES):
        b, half = core // 2, core % 2
        out[b][:, half * T:(half + 1) * T] = res.results[core]["out"]
    return out


# revision 19
# speedup vs baseline: 1.0135x; 1.0135x over previous
"""Trainium2 Bass kernel for nn_Attention_Joint_MaxPool.

Math (see reference):
  q = (Wq*scale) @ x                        (B, C, N), heads on rows
  xsr = conv2x2s2(x) ; k = Wk @ BN(xsr)     (B, C, Nk=1024)
  attn = max over keys of q_h . k_h         (B, NH, N)
  s = sum over heads of attn                (B, N)
  out[b,c,n] = (Wproj @ mean_n x)[c] * s[b,n] + bproj[c]

Weight folding done on host:
  g = gamma/sqrt(var+eps); A = Wk * g[None,:]
  k = sum_e (A @ Wsr[:,:,e]) @ x_sub[e] + ck,  ck = A@bsr + Wk@(beta-mean*g)
  pv[b] = Wproj @ mean_n x[b]   (rank-1 output structure)

Key structure (v2):
  All heavy matmuls run in bf16.  Max over keys via the pair cascade
  max(a,b) = a + relu(b-a): diff-pair matmuls write a PSUM bank, ScalarE
  relus it IN PLACE (has_written survives), even-pair matmuls accumulate
  onto it with start=False, VectorE reduce_maxes the bank.  The two heads
  of a pair run CONCURRENTLY in the PE via tile_position row pairing
  (measured: second matmul of a pair retires ~4ns after the first).

  Scheduling (the v2 changes):
  - all input DMAs ride ONE HWDGE ring in strict priority order
    (cpb -> wq+xq0+wk01 -> convA -> xq123 -> convB -> wk23) so the
    k-path data lands ~21us in instead of ~28us (round-robin starved).
  - PE warmup burst on memset junk so HAM is at 8/8 before real work.
  - k m1 accumulates in a borrowed score-pool PSUM tile during the lead.
  - passes: A (m=0), B (m=1), then CD (m=2,3 interleaved per tile), all
    with a 2-deep front/finish software pipeline so the ScalarE relu
    latency never head-of-line-blocks the PE.
  - k m2/m3 matmuls and the q-projection units are spread across pass
    A/B slots with explicit data deadlines.
  - output stage: 8 parts of 2 token tiles each, emitted as soon as the
    pair of tiles finalizes in pass CD; output DMAs on the scalar ring.

Sharding: 8 cores; core i -> batch i//2, token half i%2 (2048 tokens).
Each core is fully independent (no collectives).
"""

import os
import sys
import types
import numpy as np

# ---------------------------------------------------------------------------
# problem constants (hardcoded; kernel.py must be self-contained)
# ---------------------------------------------------------------------------
B, C, N = 4, 512, 4096
NH, HD = 8, 64
SR = 2
EPS = 1e-5
HW_ = 64                      # H = W = 64
T = N // 2                    # tokens per core
NK = 1024                     # conv output positions (keys)
NKE = NK // 2                 # even keys
MB = C // 128                 # 4 channel blocks
KC = C // 128                 # 4 contraction chunks
NCORES = 8
TT = T // 128                 # 16 token tiles per core
NCH = T // 512                # 4 q chunks per core

_cache = {}


# ---------------------------------------------------------------------------
# workarounds for this container's toolchain
# ---------------------------------------------------------------------------
def _install_fixes():
    import concourse.tile as tile
    import concourse.mybir as mybir
    from concourse.vector_clock import ScopedClock

    if getattr(tile.TileContext, "_drain_patched", False):
        return

    def _patched_drain_and_barrier(self, tick_clock, wait_clock):
        nc = self.nc
        probe = nc.sync.nop(nofuse=True, hint="drain_wait_carrier")
        wait_clock.add_sem_waits(
            probe.ins, ScopedClock({None: tick_clock.global_clock})
        )
        waits = list(probe.ins.sync_info.on_wait) if probe.ins.sync_info else []
        if len(waits) > 1:
            probe.ins.sync_info = mybir.SyncInfo(on_wait=waits[:1], on_update=[])
            for w in waits[1:]:
                extra = nc.sync.nop(nofuse=True, hint="drain_wait_carrier")
                extra.ins.sync_info = mybir.SyncInfo(on_wait=[w], on_update=[])
        nc.sync.drain()
        nc.all_engine_barrier()
        assert self.sems is not None
        popped = nc._tile_sem_poison_stack.pop()
        assert popped is self._sem_poison
        nc.clear_and_free_semaphores(list(self.sems.allocated().values()))
        nc.all_engine_barrier()

    tile.TileContext._drain_and_barrier = _patched_drain_and_barrier
    tile.TileContext._drain_patched = True


def _split_multi_waits(nc):
    """This walrus build allows only one sync-wait per instruction; hoist
    extra waits onto same-engine nops inserted just before the instruction."""
    import concourse.mybir as mybir

    ctr = 0
    for f in nc.m.functions:
        for bb in f.blocks:
            changed = False
            out = []
            for inst in bb.instructions:
                si = inst.sync_info
                tname = type(inst).__name__
                if (si is not None and si.on_wait and len(si.on_wait) > 1
                        and "Collective" not in tname):
                    waits = list(si.on_wait)
                    for w in waits[:-1]:
                        ctr += 1
                        nop = mybir.InstNoOp(
                            name=f"I-ws-{ctr}",
                            engine=inst.engine,
                            sync_info=mybir.SyncInfo(on_wait=[w], on_update=[]),
                        )
                        nc.register_instruction(nop, overwrite=True)
                        out.append(nop)
                    inst.sync_info = mybir.SyncInfo(
                        on_wait=waits[-1:], on_update=list(si.on_update)
                    )
                    changed = True
                out.append(inst)
            if changed:
                bb.instructions = out


def _install_ntff_hook():
    """Provide antenv.axon_hooks (missing in this image) so trace=True works."""
    try:
        from antenv import axon_hooks  # noqa: F401
        return
    except ImportError:
        pass
    try:
        import antenv
        from trn_agent_boot.trn_boot import _ntff_profile_via_ctypes
    except ImportError:
        return
    mod = types.ModuleType("antenv.axon_hooks")
    _hook = [None]
    mod.set_axon_ntff_profile_hook = lambda h: _hook.__setitem__(0, h)
    mod.get_axon_ntff_profile_hook = lambda: _hook[0]
    sys.modules["antenv.axon_hooks"] = mod
    antenv.axon_hooks = mod
    mod.set_axon_ntff_profile_hook(
        _ntff_profile_via_ctypes("/opt/axon/libaxon_pjrt.so")
    )


# ---------------------------------------------------------------------------
# device program
# ---------------------------------------------------------------------------
def _build_program():
    import concourse.bass as bass
    import concourse.mybir as mybir
    import concourse.tile as tile

    F32 = mybir.dt.float32
    F32R = mybir.dt.float32r
    BF16 = mybir.dt.bfloat16
    AX = mybir.AxisListType
    ACTF = mybir.ActivationFunctionType
    ALU = mybir.AluOpType

    nc = bass.Bass()

    cpbx_in = nc.declare_dram_parameter("cpbx", [128, 3 * MB], F32,
                                        isOutput=False)
    ones_in = nc.declare_dram_parameter("ones", [1, 128], F32R,
                                        isOutput=False)
    pack1_in = nc.declare_dram_parameter("pack1", [128, 8192], BF16,
                                         isOutput=False)
    convA_in = nc.declare_dram_parameter("convA", [128, 8192], BF16,
                                         isOutput=False)
    xq123_in = nc.declare_dram_parameter("xq123", [128, 6144], BF16,
                                         isOutput=False)
    convB_in = nc.declare_dram_parameter("convB", [128, 8192], BF16,
                                         isOutput=False)
    wk23_in = nc.declare_dram_parameter("wk23", [128, 4096], BF16,
                                        isOutput=False)
    out_ext = nc.declare_dram_parameter("out", [C, T], F32, isOutput=True)

    sbounce = nc.dram_tensor("sbounce", [128, TT], F32)

    # output parts: (lo_tt, hi_tt); big parts early, small parts for the tail
    PARTS = [(0, 4), (4, 8), (8, 12), (12, 14), (14, 16)]

    with tile.TileContext(nc) as tc:
        with tc.tile_pool(name="wts", bufs=1) as wts, \
             tc.tile_pool(name="xdat", bufs=1) as xdat, \
             tc.tile_pool(name="work", bufs=1) as work, \
             tc.tile_pool(name="opool", bufs=2) as opool, \
             tc.tile_pool(name="psX", bufs=3, space="PSUM") as psX, \
             tc.tile_pool(name="pkp", bufs=1, space="PSUM") as pkp:

            # ---- input DMAs ----
            # big blobs: strict priority chain on the SP HWDGE ring
            pack1 = wts.tile([128, 8192], BF16, tag="pack1")
            nc.sync.dma_start(out=pack1[:], in_=pack1_in[:])
            convA = xdat.tile([128, 8192], BF16, tag="convA")
            nc.sync.dma_start(out=convA[:], in_=convA_in[:])
            xq123 = wts.tile([128, 6144], BF16, tag="xq123")
            nc.sync.dma_start(out=xq123[:], in_=xq123_in[:])
            convB = xdat.tile([128, 8192], BF16, tag="convB")
            nc.sync.dma_start(out=convB[:], in_=convB_in[:])
            wk23 = xdat.tile([128, 4096], BF16, tag="wk23")
            nc.sync.dma_start(out=wk23[:], in_=wk23_in[:])
            # small constants ride the ACT ring in parallel
            cpbx_t = wts.tile([128, 3 * MB], F32, tag="cpbx")
            nc.scalar.dma_start(out=cpbx_t[:], in_=cpbx_in[:])
            ones = wts.tile([1, 128], F32R, tag="ones")
            nc.scalar.dma_start(out=ones[:], in_=ones_in[:])

            ck_t = cpbx_t[:, 0:MB]
            pv_t = cpbx_t[:, MB:2 * MB]
            bb_t = cpbx_t[:, 2 * MB:3 * MB]

            # warmup junk (memset; no DMA dependency)
            warm = wts.tile([128, 128], BF16, tag="warm")
            nc.gpsimd.memset(warm[:], 0.0)

            # views
            wq_t = [pack1[:, kc * 512:(kc + 1) * 512] for kc in range(KC)]
            xq_t = {}
            for kc in range(KC):
                xq_t[(0, kc)] = pack1[:, 2048 + kc * 512:2048 + (kc + 1) * 512]
                for c in range(1, NCH):
                    xq_t[(c, kc)] = xq123[:, (c - 1) * 2048 + kc * 512:
                                          (c - 1) * 2048 + (kc + 1) * 512]
            xce_t, xcd_t, wksr_t = {}, {}, {}
            for kc in range(KC):
                blob = convA if kc < 2 else convB
                base = (kc % 2) * 4096
                for e in range(4):
                    xce_t[(e, kc)] = blob[:, base + e * 512:
                                          base + (e + 1) * 512]
                    xcd_t[(e, kc)] = blob[:, base + 2048 + e * 512:
                                          base + 2048 + (e + 1) * 512]
                for m in range(MB):
                    if m < 2:
                        wksr_t[(m, kc)] = pack1[:, 4096 + m * 2048 + kc * 512:
                                                4096 + m * 2048 + (kc + 1) * 512]
                    else:
                        wksr_t[(m, kc)] = wk23[:, (m - 2) * 2048 + kc * 512:
                                               (m - 2) * 2048 + (kc + 1) * 512]

            # ---- persistent activations ----
            q_sb = [work.tile([128, T], BF16, tag=f"q{m}", name=f"q{m}")
                    for m in range(MB)]
            k2_sb = [work.tile([128, NK], BF16, tag=f"k2{m}", name=f"k2{m}")
                     for m in range(MB)]
            s_acc = work.tile([128, TT * NH], F32, tag="sacc")
            s_cols = work.tile([128, TT], F32, tag="scols")
            sflat = work.tile([1, T], F32R, tag="sflat")

            # ---- q projection unit for one (chunk, head-pair) ----
            def emit_q_unit(c, m, eng="v"):
                pq = psX.tile([128, 1024], F32, tag="xbank",
                              name=f"pq{c}_{m}")
                for kc in range(KC):
                    nc.tensor.matmul(
                        pq[:, 0:512],
                        wq_t[kc][:, m * 128:(m + 1) * 128],
                        xq_t[(c, kc)],
                        start=(kc == 0), stop=(kc == KC - 1))
                dst = q_sb[m][:, c * 512:(c + 1) * 512]
                # engine split chosen to balance Scalar vs Vector per phase
                if eng == "s":
                    nc.scalar.copy(dst, pq[:, 0:512])
                else:
                    nc.vector.tensor_copy(dst, pq[:, 0:512])

            # ---- k even/diff banks for head pair m ----
            def emit_k_mms(m, pk, units):
                for (e, kc) in units:
                    first = (kc == 0 and e == 0)
                    last = (kc == KC - 1 and e == 3)
                    nc.tensor.matmul(
                        pk[:, 0:512],
                        wksr_t[(m, kc)][:, e * 128:(e + 1) * 128],
                        xce_t[(e, kc)],
                        start=first, stop=last)
                    nc.tensor.matmul(
                        pk[:, 512:1024],
                        wksr_t[(m, kc)][:, e * 128:(e + 1) * 128],
                        xcd_t[(e, kc)],
                        start=first, stop=last)

            def emit_k_act(m, pk):
                nc.scalar.activation(
                    k2_sb[m][:, 0:512], pk[:, 0:512], ACTF.Identity,
                    bias=ck_t[:, m:m + 1], scale=1.0)
                nc.scalar.copy(k2_sb[m][:, 512:1024], pk[:, 512:1024])

            # ---- score groups: front (diff+relu) / finish (even+reduce) ----
            state = {}

            def emit_front(m, tt):
                tsl = slice(tt * 128, (tt + 1) * 128)
                qs = q_sb[m]
                pX = psX.tile([128, 1024], F32, tag="xbank",
                              name=f"pX{m}_{tt}")
                nc.tensor.matmul(pX[:, 0:512], qs[0:64, tsl],
                                 k2_sb[m][0:64, 512:1024], start=True,
                                 stop=True, tile_position=(0, 0))
                nc.tensor.matmul(pX[:, 512:1024], qs[64:128, tsl],
                                 k2_sb[m][64:128, 512:1024], start=True,
                                 stop=True, tile_position=(64, 0))
                nc.scalar.activation(pX[:], pX[:], ACTF.Relu)
                state[(m, tt)] = pX

            def emit_finish(m, tt):
                tsl = slice(tt * 128, (tt + 1) * 128)
                qs = q_sb[m]
                pX = state.pop((m, tt))
                nc.tensor.matmul(pX[:, 0:512], qs[0:64, tsl],
                                 k2_sb[m][0:64, 0:512], start=False,
                                 stop=True, tile_position=(0, 0))
                nc.tensor.matmul(pX[:, 512:1024], qs[64:128, tsl],
                                 k2_sb[m][64:128, 0:512], start=False,
                                 stop=True, tile_position=(64, 0))
                cols = slice(tt * NH + 2 * m, tt * NH + 2 * m + 2)
                nc.vector.reduce_max(
                    s_acc[:, cols],
                    pX[:].rearrange("p (a b) -> p a b", a=2), axis=AX.X)
                if m == MB - 1:
                    nc.vector.reduce_sum(
                        s_cols[:, tt:tt + 1],
                        s_acc[:, tt * NH:(tt + 1) * NH], axis=AX.X)

            # ---- rank-1 output stage, one part = PARTS[p] token tiles ----
            # s_cols -> DRAM bounce -> token-major sflat (GpSimd gather),
            # then a ones-matmul broadcasts s over the 128 channel
            # partitions.  The four steps are staged one score-group apart
            # so the DMA latency never head-of-line-blocks the PE.
            out_state = {}

            def outer_g1(p):
                lo_tt, hi_tt = PARTS[p]
                with nc.named_scope("outer"):
                    nc.sync.dma_start(out=sbounce[:, lo_tt:hi_tt],
                                      in_=s_cols[:, lo_tt:hi_tt])

            def outer_g2(p):
                lo_tt, hi_tt = PARTS[p]
                with nc.named_scope("outer"):
                    nc.gpsimd.dma_start(
                        out=sflat[0:1, lo_tt * 128:hi_tt * 128],
                        in_=sbounce[:, lo_tt:hi_tt].rearrange(
                            "p t -> () t p"))

            def outer_mm(p):
                lo_tt, hi_tt = PARTS[p]
                ntok = (hi_tt - lo_tt) * 128
                with nc.named_scope("outer"):
                    pbc = pkp.tile([128, 1024], F32, tag="kbank",
                                   name=f"pbc{p}")
                    nc.tensor.matmul(
                        pbc[:, 0:ntok], ones[:],
                        sflat[0:1, lo_tt * 128:hi_tt * 128],
                        start=True, stop=True)
                    out_state[p] = pbc

            def outer_acts(p):
                lo_tt, hi_tt = PARTS[p]
                ntok = (hi_tt - lo_tt) * 128
                tok = slice(lo_tt * 128, hi_tt * 128)
                pbc = out_state.pop(p)
                last = (p == len(PARTS) - 1)
                with nc.named_scope("outer"):
                    osb = opool.tile([128, 4 * 512], F32, tag="osb",
                                     name=f"osb{p}")
                    for m in range(MB):
                        if last and m % 2 == 1:
                            nc.vector.tensor_scalar(
                                osb[:, m * ntok:(m + 1) * ntok],
                                pbc[:, 0:ntok],
                                pv_t[:, m:m + 1], bb_t[:, m:m + 1],
                                op0=ALU.mult, op1=ALU.add)
                        else:
                            nc.scalar.activation(
                                osb[:, m * ntok:(m + 1) * ntok],
                                pbc[:, 0:ntok], ACTF.Identity,
                                bias=bb_t[:, m:m + 1], scale=pv_t[:, m:m + 1])
                    nc.scalar.dma_start(
                        out=out_ext[:, tok].rearrange("(m p) t -> p m t",
                                                      m=MB),
                        in_=osb[:, 0:MB * ntok].rearrange(
                            "p (m t) -> p m t", m=MB))

            # ------------------ emission schedule ------------------
            units_of = lambda kcs: [(e, kc) for kc in kcs for e in range(4)]

            # warmup: ~36 junk MMs to trip HAM to 8/8 before real work
            pkw = pkp.tile([128, 1024], F32, tag="kbank", name="pkwarm")
            with nc.named_scope("warm"):
                for i in range(36):
                    nc.tensor.matmul(pkw[:, 0:128], warm[:], warm[:],
                                     start=True, stop=True)

            # lead
            with nc.named_scope("lead"):
                for m in range(MB):
                    emit_q_unit(0, m)          # pack1 (q chunk 0, all pairs)
                pk0 = pkp.tile([128, 1024], F32, tag="kbank", name="pk0")
                xbm1 = psX.tile([128, 1024], F32, tag="xbank", name="xbm1")
                # convA: kc0/1 for m0 and m1, interleaved
                for kc in (0, 1):
                    emit_k_mms(0, pk0, units_of([kc]))
                    emit_k_mms(1, xbm1, units_of([kc]))
                emit_q_unit(1, 0)              # xq123
                emit_q_unit(1, 1)
                # convB: m1 first so k_act(1) frees xbm1's pool slot before
                # pass A's third front needs it
                for kc in (2, 3):
                    emit_k_mms(1, xbm1, units_of([kc]))
                emit_k_act(1, xbm1)
                for kc in (2, 3):
                    emit_k_mms(0, pk0, units_of([kc]))
                emit_k_act(0, pk0)

            # score passes with 2-deep pipeline
            pending = []

            def do_group(m, tt):
                emit_front(m, tt)
                pending.append((m, tt))
                if len(pending) > 2:
                    emit_finish(*pending.pop(0))

            def drain():
                while pending:
                    emit_finish(*pending.pop(0))

            # pass A: m=0; fillers: k m2 (kc0..3), q units (scalar casts:
            # ScalarE has slack in A/B, VectorE is the tighter engine)
            pkA = pkp.tile([128, 1024], F32, tag="kbank", name="pk2")
            fillA = {
                1: lambda: emit_k_mms(2, pkA, units_of([0])[0:2]),
                2: lambda: emit_k_mms(2, pkA, units_of([0])[2:4]),
                3: lambda: emit_q_unit(2, 0, "s"),
                4: lambda: emit_k_mms(2, pkA, units_of([1])[0:2]),
                5: lambda: emit_k_mms(2, pkA, units_of([1])[2:4]),
                6: lambda: emit_q_unit(3, 0, "s"),
                7: lambda: emit_k_mms(2, pkA, units_of([2])[0:2]),
                8: lambda: emit_k_mms(2, pkA, units_of([2])[2:4]),
                9: lambda: emit_q_unit(1, 2, "s"),
                10: lambda: emit_k_mms(2, pkA, units_of([3])[0:2]),
                11: lambda: (emit_k_mms(2, pkA, units_of([3])[2:4]),
                             emit_k_act(2, pkA)),
            }
            with nc.named_scope("passA"):
                for tt in range(TT):
                    do_group(0, tt)
                    f = fillA.get(tt)
                    if f:
                        f()

            # pass B: m=1; fillers: k m3 + the pass-B-deadline q units
            pkB = pkp.tile([128, 1024], F32, tag="kbank", name="pk3")
            fillB = {
                1: lambda: emit_k_mms(3, pkB, units_of([0])[0:2]),
                2: lambda: emit_k_mms(3, pkB, units_of([0])[2:4]),
                3: lambda: emit_q_unit(2, 1, "s"),
                4: lambda: emit_k_mms(3, pkB, units_of([1])[0:2]),
                5: lambda: emit_k_mms(3, pkB, units_of([1])[2:4]),
                6: lambda: emit_q_unit(3, 1, "s"),
                7: lambda: emit_k_mms(3, pkB, units_of([2])[0:2]),
                8: lambda: emit_k_mms(3, pkB, units_of([2])[2:4]),
                9: lambda: emit_q_unit(1, 3, "s"),
                10: lambda: emit_k_mms(3, pkB, units_of([3])[0:2]),
                11: lambda: (emit_k_mms(3, pkB, units_of([3])[2:4]),
                             emit_k_act(3, pkB)),
            }
            with nc.named_scope("passB"):
                for tt in range(TT):
                    do_group(1, tt)
                    f = fillB.get(tt)
                    if f:
                        f()

            # pass CD: m=2,3 interleaved per tile; output-part work is queued
            # at each part's final finish and drained one action per group so
            # nothing bunches up or head-of-line-blocks the PE.
            fin_hooks = {}
            actions = []

            def do_group_cd(m, tt):
                emit_front(m, tt)
                pending.append((m, tt))
                if len(pending) > 2:
                    g = pending.pop(0)
                    emit_finish(*g)
                    h = fin_hooks.pop(g, None)
                    if h:
                        actions.extend(h)
                if m == 2 and actions:
                    actions.pop(0)()

            with nc.named_scope("passCD"):
                for p in range(len(PARTS)):
                    fin_hooks[(3, PARTS[p][1] - 1)] = [
                        (lambda pp: lambda: outer_g1(pp))(p),
                        (lambda pp: lambda: outer_g2(pp))(p),
                        (lambda pp: lambda: outer_mm(pp))(p),
                        (lambda pp: lambda: outer_acts(pp))(p),
                    ]
                qcd = {0: (2, 2), 2: (2, 3), 6: (3, 2), 8: (3, 3)}
                for tt in range(TT):
                    do_group_cd(2, tt)
                    if tt in qcd:
                        emit_q_unit(*qcd[tt])
                    do_group_cd(3, tt)
                while pending:
                    g = pending.pop(0)
                    emit_finish(*g)
                    h = fin_hooks.pop(g, None)
                    if h:
                        actions.extend(h)
                    if actions:
                        actions.pop(0)()
                while actions:
                    actions.pop(0)()

    _split_multi_waits(nc)
    return nc


# ---------------------------------------------------------------------------
# host side
# ---------------------------------------------------------------------------
def _prep_host(x, Wq, Wk, Wsr, bsr, bn_gamma, bn_beta, bn_mean, bn_var,
               Wproj, bproj):
    import ml_dtypes
    bf16 = ml_dtypes.bfloat16
    f8 = np.float64
    scale = HD ** -0.5
    g = bn_gamma.astype(f8) / np.sqrt(bn_var.astype(f8) + EPS)
    A = Wk.astype(f8) * g[None, :]
    ck = A @ bsr.astype(f8) + Wk.astype(f8) @ (
        bn_beta.astype(f8) - bn_mean.astype(f8) * g)
    # k weights: wk4[e] = (A @ Wsr[:,:,e]).T   (C_in, C_out)
    wk4 = np.stack([
        (A @ Wsr[:, :, e // 2, e % 2].astype(f8)).T for e in range(4)
    ])
    # wkblk[m][kc][e] = wk4[e][kc*128:(kc+1)*128, m*128:(m+1)*128]
    def wkblk(m):
        cols = np.empty((128, 2048), np.float64)
        for kc in range(KC):
            for e in range(4):
                cols[:, kc * 512 + e * 128:kc * 512 + (e + 1) * 128] = \
                    wk4[e][kc * 128:(kc + 1) * 128, m * 128:(m + 1) * 128]
        return cols

    wqT = (Wq.astype(f8) * scale).T                    # (C_in, C_out)

    x4 = x.reshape(B, C, HW_, HW_)
    xce = np.empty((B, C, 4, NKE), np.float32)
    xcd = np.empty((B, C, 4, NKE), np.float32)
    for e in range(4):
        di, dj = e // 2, e % 2
        even = x4[:, :, di::2, dj::4].reshape(B, C, NKE)
        odd = x4[:, :, di::2, dj + 2::4].reshape(B, C, NKE)
        xce[:, :, e] = even
        xcd[:, :, e] = odd - even
    xce = xce.reshape(B, C, 4 * NKE)
    xcd = xcd.reshape(B, C, 4 * NKE)
    # convA/convB [128, 8192]: [kc-of-pair][even|diff][e][512]
    convA = np.empty((B, 128, 8192), np.float32)
    convB = np.empty((B, 128, 8192), np.float32)
    for kc in range(KC):
        blob = convA if kc < 2 else convB
        base = (kc % 2) * 4096
        rows = slice(kc * 128, (kc + 1) * 128)
        blob[:, :, base:base + 2048] = xce[:, rows]
        blob[:, :, base + 2048:base + 4096] = xcd[:, rows]
    convA = convA.astype(bf16)
    convB = convB.astype(bf16)

    v = x.astype(f8).mean(axis=2)                       # (B, C)
    pv = (Wproj.astype(f8) @ v.T).T.astype(np.float32)  # (B, C)

    ck_t = ck.astype(np.float32).reshape(MB, 128).T    # (128, MB)
    bb_t = bproj.astype(np.float32).reshape(MB, 128).T
    cpbx = [np.concatenate(
        [ck_t, pv[b].reshape(MB, 128).T, bb_t], axis=1).astype(np.float32)
        for b in range(B)]                              # (128, 3*MB)

    # pack1 [128, 8192]: wq kc-major (2048) | xq chunk0 (2048) | wk m0,m1
    # xq123 [128, 6144]: [c-1][kc][512]
    # wk23  [128, 4096]: [m-2][kc][e][128]
    pack1 = np.empty((B, 2, 128, 8192), np.float32)
    xq123 = np.empty((B, 2, 128, 6144), np.float32)
    for half in range(2):
        xh = x[:, :, half * T:(half + 1) * T]
        for kc in range(KC):
            rows = slice(kc * 128, (kc + 1) * 128)
            pack1[:, half, :, kc * 512:(kc + 1) * 512] = wqT[None, rows]
            pack1[:, half, :, 2048 + kc * 512:2048 + (kc + 1) * 512] = \
                xh[:, rows, 0:512]
            for c in range(1, NCH):
                xq123[:, half, :, (c - 1) * 2048 + kc * 512:
                      (c - 1) * 2048 + (kc + 1) * 512] = \
                    xh[:, rows, c * 512:(c + 1) * 512]
    pack1[:, :, :, 4096:6144] = wkblk(0)[None, None]
    pack1[:, :, :, 6144:8192] = wkblk(1)[None, None]
    wk23 = np.concatenate([wkblk(2), wkblk(3)], axis=1).astype(bf16)
    pack1 = pack1.astype(bf16)
    xq123 = xq123.astype(bf16)
    return cpbx, pack1, convA, xq123, convB, wk23


def kernel(x, y, Wq, Wk, Wsr, bsr, bn_gamma, bn_beta, bn_mean, bn_var,
           Wproj, bproj, H, W):
    x = np.asarray(x, np.float32)
    cpbx, pack1, convA, xq123, convB, wk23 = _prep_host(
        x, np.asarray(Wq, np.float32), np.asarray(Wk, np.float32),
        np.asarray(Wsr, np.float32), np.asarray(bsr, np.float32),
        np.asarray(bn_gamma, np.float32), np.asarray(bn_beta, np.float32),
        np.asarray(bn_mean, np.float32), np.asarray(bn_var, np.float32),
        np.asarray(Wproj, np.float32), np.asarray(bproj, np.float32))

    _install_fixes()
    _install_ntff_hook()
    from concourse.bass_utils import run_bass_kernel_spmd

    if "nc" not in _cache:
        _cache["nc"] = _build_program()
    nc = _cache["nc"]

    in_maps = []
    for core in range(NCORES):
        b, half = core // 2, core % 2
        in_maps.append({
            "cpbx": cpbx[b],
            "pack1": pack1[b, half],
            "convA": convA[b],
            "xq123": xq123[b, half],
            "convB": convB[b],
            "wk23": wk23,
            "ones": np.ones((1, 128), np.float32),
        })

    trace = os.environ.get("BASS_KERNEL_TRACE", "0") == "1"
    res = run_bass_kernel_spmd(nc, in_maps, list(range(NCORES)), trace=trace)
    if trace:
        print(f"HW exec time: {res.exec_time_ns} ns")
        _cache["last_exec_time_ns"] = res.exec_time_ns
        _cache["last_trace"] = res.instructions_and_trace

    out = np.empty((B, C, N), np.float32)
    for core in range(NCORES):
        b, half = core // 2, core % 2
        out[b][:, half * T:(half + 1) * T] = res.results[core]["out"]
    return out
